# revision 5
# baseline (speedup 1.0000x reference)
"""BlipAttention kernel for 8 Trainium2 NeuronCores.

Strategy: data-parallel over batch (16 batches -> 2 per core), no collectives.
Per core: fused QKV projection + 16-head scaled-dot-product attention + output
projection on the PE, bf16 matmuls with fp32 PSUM accumulation.

Layout tricks:
  - x is transposed on-chip (PE transpose) to feature-major x^T so the
    contraction dim (D) lives on SBUF partitions for all projections.
  - scores are computed TRANSPOSED (k-tokens on partitions) so softmax
    denominators come for free from the PV matmul: v is stored token-major
    with ones-columns appended per head (97-wide groups, cols 88..96 = 1.0),
    which makes the PV matmul emit  sum_k exp(scores)  at PSUM partition 96
    (a legal quadrant offset for the subsequent reciprocal read).
  - 1/denom is broadcast across partitions with a rank-1 (K=1) matmul.
  - biases are applied via ACT bias (feature-major q,k) or rank-1 ones
    matmuls (token-major v / output projection).
  - weights are host-converted to bf16 and DMA'd in batched stripes on the
    otherwise-idle GpSimd DMA queue.
"""

import contextlib

import numpy as np
import ml_dtypes

import concourse.bass as bass
import concourse.tile as tile
from concourse import bacc, mybir
from concourse.bass_utils import run_bass_kernel_spmd
from concourse.masks import make_identity

F32 = mybir.dt.float32
F32R = mybir.dt.float32r
BF16 = mybir.dt.bfloat16

N_CORES = 8
B_TOTAL, S, D = 16, 577, 1408
H, HD = 16, 88
SCALE = HD ** -0.5
B = B_TOTAL // N_CORES          # batches per core = 2
T = B * S                       # tokens per core = 1154
SP = S + 1                      # padded per-batch token span = 578
KT = D // 128                   # 11 k-tiles over D
TT = (S + 127) // 128           # 5 token tiles per batch (128,128,128,128,65)
VG = 97                         # v group width per head: 88 v cols + 9 ones
DEN = 96                        # psum partition of the softmax denominator

# moving-dim chunks for 578-wide spans: (logical col, psum col, width)
CH_S = [(0, 0, 512), (512, 512, 66)]
# chunks for 1408-wide spans (each fits one psum bank)
CH_D = [(0, 512), (512, 512), (1024, 384)]


def _tok_tiles():
    out = []
    for tt in range(TT):
        t0 = tt * 128
        out.append((tt, t0, min(128, S - t0)))
    return out


def build_program():
    nc = bacc.Bacc("TRN2", target_bir_lowering=False, debug=False,
                   num_devices=N_CORES)

    x_ap = nc.dram_tensor("x", [T, D], F32, kind="ExternalInput").ap()
    wqkv_ap = nc.dram_tensor("w_qkv_bf", [D, 3 * D], BF16, kind="ExternalInput").ap()
    bq_col_ap = nc.dram_tensor("b_qkv_col", [2 * D, 1], F32, kind="ExternalInput").ap()
    bv_row_ap = nc.dram_tensor("b_v_row", [1, D], BF16, kind="ExternalInput").ap()
    wp_ap = nc.dram_tensor("w_proj_bf", [D, D], BF16, kind="ExternalInput").ap()
    bp_row_ap = nc.dram_tensor("b_proj_row", [1, D], BF16, kind="ExternalInput").ap()
    ones_ap = nc.dram_tensor("ones_f", [1, 128], F32, kind="ExternalInput").ap()
    ones_bf_ap = nc.dram_tensor("ones_bf", [128, 128], BF16, kind="ExternalInput").ap()
    out_ap = nc.dram_tensor("out", [T, D], F32, kind="ExternalOutput").ap()

    with tile.TileContext(nc) as tc, contextlib.ExitStack() as ctx:
        p_xraw = ctx.enter_context(tc.tile_pool(name="xraw", bufs=2))
        p_xT = ctx.enter_context(tc.tile_pool(name="xT", bufs=11))
        p_vsb = ctx.enter_context(tc.tile_pool(name="vsb", bufs=5))
        p_qk = ctx.enter_context(tc.tile_pool(name="qk", bufs=8))
        p_expT = ctx.enter_context(tc.tile_pool(name="expT", bufs=8))
        p_pvsb = ctx.enter_context(tc.tile_pool(name="pvsb", bufs=3))
        p_attn = ctx.enter_context(tc.tile_pool(name="attn", bufs=18))
        p_rec = ctx.enter_context(tc.tile_pool(name="rec", bufs=3))
        p_wq = ctx.enter_context(tc.tile_pool(name="wq", bufs=5))
        p_wv = ctx.enter_context(tc.tile_pool(name="wv", bufs=13))
        p_wp = ctx.enter_context(tc.tile_pool(name="wp", bufs=17))
        p_bias = ctx.enter_context(tc.tile_pool(name="bias", bufs=8))
        p_brow = ctx.enter_context(tc.tile_pool(name="brow", bufs=2))
        p_const = ctx.enter_context(tc.tile_pool(name="const", bufs=1))
        p_ost = ctx.enter_context(tc.tile_pool(name="ost", bufs=3))

        ps_wide = ctx.enter_context(tc.tile_pool(name="pswide", bufs=3, space="PSUM"))
        ps_proj = ctx.enter_context(tc.tile_pool(name="psproj", bufs=2, space="PSUM"))

        ident = p_const.tile([128, 128], F32, tag="ident")
        make_identity(nc, ident[:])
        ones = p_const.tile([1, 128], F32R, tag="ones")
        nc.sync.dma_start(ones[:], ones_ap[0:1, :].bitcast(F32R))
        ones_bf = p_const.tile([1, 128], BF16, tag="ones_bf")
        nc.sync.dma_start(ones_bf[:], ones_bf_ap[0:1, :])

        bvr = p_brow.tile([1, D], BF16, tag="bvr")
        nc.sync.dma_start(bvr[:], bv_row_ap[:])
        bpr = p_brow.tile([1, D], BF16, tag="bpr")
        nc.sync.dma_start(bpr[:], bp_row_ap[:])

        # x^T tiles cover both batches; allocated once, written per batch.
        xT = [p_xT.tile([128, B * SP], BF16, tag="xT", name=f"xT{k}")
              for k in range(KT)]

        for b in range(B):
            boff = b * SP

            # ---- stage A: load x (token-major) and transpose to x^T ----
            for tt, t0, ts in _tok_tiles():
                xr = p_xraw.tile([128, D], F32, tag="xraw")
                nc.sync.dma_start(xr[0:ts, :], x_ap[b * S + t0: b * S + t0 + ts, :])
                for k in range(KT):
                    pt = ps_wide.tile([128, 1024], F32, tag="ps")
                    nc.tensor.transpose(pt[0:128, 0:ts], xr[0:ts, k * 128:(k + 1) * 128],
                                        ident[0:ts, 0:ts])
                    nc.vector.tensor_copy(xT[k][:, boff + t0: boff + t0 + ts],
                                          pt[0:128, 0:ts])
            # fill padded token column (keeps downstream values finite)
            for k in range(KT):
                nc.sync.dma_start(xT[k][:, boff + S: boff + S + 1],
                                  ones_bf_ap[:, 0:1])

            # ---- stage B: v projection, token-major, head-interleaved ----
            vsb = []
            for tt, t0, ts in _tok_tiles():
                vt = p_vsb.tile([128, H * VG], BF16, tag="vsb")
                vsb.append(vt)
                nc.vector.memset(vt[:], 1.0)  # ones-columns (+ unused rows)
            for (c0, w) in CH_D:
                wvs = []
                for k in range(KT):
                    wv = p_wv.tile([128, 512], BF16, tag="wv")
                    nc.gpsimd.dma_start(
                        wv[:, 0:w],
                        wqkv_ap[k * 128:(k + 1) * 128, 2 * D + c0: 2 * D + c0 + w])
                    wvs.append(wv)
                for tt, t0, ts in _tok_tiles():
                    pv = ps_proj.tile([128, 512], F32, tag="ps1")
                    for k in range(KT):
                        nc.tensor.matmul(pv[0:ts, 0:w],
                                         xT[k][:, boff + t0: boff + t0 + ts],
                                         wvs[k][:, 0:w], start=(k == 0), stop=False)
                    nc.tensor.matmul(pv[0:ts, 0:w], ones_bf[:, 0:ts],
                                     bvr[:, c0:c0 + w], start=False, stop=True)
                    # split per head into the 97-wide groups
                    h0 = c0 // HD
                    h1 = min(H - 1, (c0 + w - 1) // HD)
                    for h in range(h0, h1 + 1):
                        s0 = max(c0, h * HD)
                        s1 = min(c0 + w, (h + 1) * HD)
                        if s1 <= s0:
                            continue
                        nc.vector.tensor_copy(
                            vsb[tt][0:ts, h * VG + (s0 - h * HD): h * VG + (s1 - h * HD)],
                            pv[0:ts, s0 - c0: s1 - c0])

            # ---- stage C: per-head attention ----
            attn = []
            for h in range(H):
                # q,k per-head feature-major projections (M=88);
                # weight stripe for all 11 k-tiles arrives as one DMA.
                qh = p_qk.tile([HD, SP], BF16, tag="qk")
                kh = p_qk.tile([HD, SP], BF16, tag="qk")
                for which, dst in ((0, qh), (1, kh)):
                    col = which * D + h * HD
                    wq = p_wq.tile([128, KT * HD], BF16, tag="wq")
                    nc.gpsimd.dma_start(
                        wq[:].rearrange("p (k c) -> p k c", k=KT),
                        wqkv_ap[:, col: col + HD].rearrange(
                            "(k p) c -> p k c", p=128))
                    pt = ps_wide.tile([128, 1024], F32, tag="ps")
                    for k in range(KT):
                        for (lc, pc, w) in CH_S:
                            nc.tensor.matmul(pt[0:HD, pc:pc + w],
                                             wq[:, k * HD:(k + 1) * HD],
                                             xT[k][:, boff + lc: boff + lc + w],
                                             start=(k == 0), stop=(k == KT - 1))
                    bq = p_bias.tile([HD, 1], F32, tag="bias")
                    nc.sync.dma_start(bq[:], bq_col_ap[col: col + HD, :])
                    for (lc, pc, w) in CH_S:
                        nc.scalar.activation(dst[:, lc:lc + w], pt[0:HD, pc:pc + w],
                                             mybir.ActivationFunctionType.Identity,
                                             bias=bq[:])

                # transposed scores + exp, per k-token tile
                expT = []
                for tt, t0, ts in _tok_tiles():
                    pt = ps_wide.tile([128, 1024], F32, tag="ps")
                    for (lc, pc, w) in CH_S:
                        nc.tensor.matmul(pt[0:ts, pc:pc + w],
                                         kh[:, t0:t0 + ts], qh[:, lc:lc + w],
                                         start=True, stop=True)
                    et = p_expT.tile([128, SP], BF16, tag="expT")
                    expT.append(et)
                    for (lc, pc, w) in CH_S:
                        nc.scalar.activation(et[0:ts, lc:lc + w], pt[0:ts, pc:pc + w],
                                             mybir.ActivationFunctionType.Exp,
                                             scale=SCALE)

                # PV with fused denominator at psum partition 96
                pv = ps_wide.tile([VG, 1024], F32, tag="ps")
                for tt, t0, ts in _tok_tiles():
                    for (lc, pc, w) in CH_S:
                        nc.tensor.matmul(pv[0:VG, pc:pc + w],
                                         vsb[tt][0:ts, h * VG:(h + 1) * VG],
                                         expT[tt][0:ts, lc:lc + w],
                                         start=(tt == 0), stop=(tt == TT - 1))

                rec = p_rec.tile([1, SP], F32R, tag="rec")
                with nc.allow_low_precision(reason="softmax reciprocal"):
                    for (lc, pc, w) in CH_S:
                        nc.vector.reciprocal(rec[:, lc:lc + w],
                                             pv[DEN:DEN + 1, pc:pc + w])
                pvs = p_pvsb.tile([HD, SP], F32, tag="pvsb")
                for (lc, pc, w) in CH_S:
                    nc.scalar.activation(pvs[:, lc:lc + w], pv[0:HD, pc:pc + w],
                                         mybir.ActivationFunctionType.Copy)
                # broadcast 1/denom over partitions via rank-1 matmul
                pb = ps_wide.tile([HD, 1024], F32, tag="ps")
                for (lc, pc, w) in CH_S:
                    nc.tensor.matmul(pb[0:HD, pc:pc + w], ones[:, 0:HD],
                                     rec[:, lc:lc + w], start=True, stop=True)
                at = p_attn.tile([HD, SP], BF16, tag="attn")
                for (lc, pc, w) in CH_S:
                    nc.vector.tensor_mul(at[:, lc:lc + w], pvs[:, lc:lc + w],
                                         pb[0:HD, pc:pc + w])
                attn.append(at)

            # ---- stage D: output projection (token-major, K=88 pieces) ----
            for (c0, w) in CH_D:
                wps = []
                for h in range(H):
                    wpt = p_wp.tile([HD, 512], BF16, tag="wp")
                    nc.gpsimd.dma_start(wpt[:, 0:w],
                                        wp_ap[h * HD:(h + 1) * HD, c0:c0 + w])
                    wps.append(wpt)
                for tt, t0, ts in _tok_tiles():
                    po = ps_proj.tile([128, 512], F32, tag="ps1")
                    for h in range(H):
                        nc.tensor.matmul(po[0:ts, 0:w], attn[h][:, t0:t0 + ts],
                                         wps[h][:, 0:w], start=(h == 0), stop=False)
                    nc.tensor.matmul(po[0:ts, 0:w], ones_bf[:, 0:ts],
                                     bpr[:, c0:c0 + w], start=False, stop=True)
                    ot = p_ost.tile([128, 512], F32, tag="ost")
                    nc.scalar.activation(ot[0:ts, 0:w], po[0:ts, 0:w],
                                         mybir.ActivationFunctionType.Copy)
                    nc.sync.dma_start(
                        out_ap[b * S + t0: b * S + t0 + ts, c0:c0 + w], ot[0:ts, 0:w])

    nc.compile()
    return nc


_NC_CACHE = None


def _get_nc():
    global _NC_CACHE
    if _NC_CACHE is None:
        _NC_CACHE = build_program()
    return _NC_CACHE


def make_in_maps(hidden_states, w_qkv, b_qkv, w_proj, b_proj):
    hidden_states = np.asarray(hidden_states, dtype=np.float32)
    w_qkv = np.ascontiguousarray(np.asarray(w_qkv, dtype=np.float32))
    b_qkv = np.asarray(b_qkv, dtype=np.float32)
    w_proj = np.asarray(w_proj, dtype=np.float32)
    b_proj = np.asarray(b_proj, dtype=np.float32)

    wqkv_bf = w_qkv.astype(ml_dtypes.bfloat16)
    wp_bf = w_proj.astype(ml_dtypes.bfloat16)
    bq_col = b_qkv[: 2 * D].reshape(2 * D, 1).copy()
    bv_row = b_qkv[2 * D:].astype(ml_dtypes.bfloat16).reshape(1, D).copy()
    bp_row = b_proj.astype(ml_dtypes.bfloat16).reshape(1, D).copy()
    ones_f = np.ones((1, 128), np.float32)
    ones_bf = np.ones((128, 128), ml_dtypes.bfloat16)

    in_maps = []
    for c in range(N_CORES):
        xs = hidden_states[c * B:(c + 1) * B].reshape(T, D)
        in_maps.append({
            "x": np.ascontiguousarray(xs),
            "w_qkv_bf": wqkv_bf,
            "b_qkv_col": bq_col,
            "b_v_row": bv_row,
            "w_proj_bf": wp_bf,
            "b_proj_row": bp_row,
            "ones_f": ones_f,
            "ones_bf": ones_bf,
        })
    return in_maps


def kernel(hidden_states, w_qkv, b_qkv, w_proj, b_proj):
    nc = _get_nc()
    in_maps = make_in_maps(hidden_states, w_qkv, b_qkv, w_proj, b_proj)
    res = run_bass_kernel_spmd(nc, in_maps, list(range(N_CORES)))
    out = np.concatenate(
        [res.results[c]["out"].reshape(B, S, D) for c in range(N_CORES)], axis=0)
    return out.astype(np.float32)


if __name__ == "__main__":
    rng = np.random.default_rng(0)
    hs = rng.standard_normal((B_TOTAL, S, D), dtype=np.float32)
    wq = rng.standard_normal((D, 3 * D), dtype=np.float32) * D ** -0.5
    bq = rng.standard_normal(3 * D).astype(np.float32) * 0.02
    wp = rng.standard_normal((D, D), dtype=np.float32) * D ** -0.5
    bp = rng.standard_normal(D).astype(np.float32) * 0.02
    o = kernel(hidden_states=hs, w_qkv=wq, b_qkv=bq, w_proj=wp, b_proj=bp)
    print(o.shape, o.dtype)


# revision 7
# speedup vs baseline: 1.3640x; 1.3640x over previous
"""BlipAttention kernel for 8 Trainium2 NeuronCores.

Strategy: data-parallel over batch (16 batches -> 2 per core), no collectives.
Per core: fused QKV projection + 16-head scaled-dot-product attention + output
projection on the PE, bf16 matmuls with fp32 PSUM accumulation.

Layout tricks:
  - x is transposed on-chip (PE transpose) to feature-major x^T so the
    contraction dim (D) lives on SBUF partitions for all projections.
  - q,k are projected with full 128-wide M tiles (feature-packed), then
    redistributed to per-head [88, S] tiles with SBUF->SBUF DMAs (DMA can
    shift partition offsets; compute engines cannot).
  - scores are computed TRANSPOSED (k-tokens on partitions) so softmax
    denominators come for free from the PV matmul: v is stored token-major
    with ones-columns appended per head (97-wide groups, cols 88..96 = 1.0),
    which makes the PV matmul emit  sum_k exp(scores)  at PSUM partition 96
    (a legal quadrant offset for the subsequent reciprocal read).
  - 1/denom is broadcast across partitions with a rank-1 (K=1) matmul.
  - attention outputs are DMA-packed back to 128-wide K tiles so the output
    projection contracts with K=128 pieces.
  - biases are applied via ACT bias (feature-major q,k) or rank-1 ones
    matmuls (token-major v / output projection).
  - weights are host-converted to bf16 and DMA'd in batched stripes on the
    otherwise-idle GpSimd DMA queue.
"""

import contextlib

import numpy as np
import ml_dtypes

import concourse.bass as bass
import concourse.tile as tile
from concourse import bacc, mybir
from concourse.bass_utils import run_bass_kernel_spmd
from concourse.masks import make_identity

F32 = mybir.dt.float32
F32R = mybir.dt.float32r
BF16 = mybir.dt.bfloat16

N_CORES = 8
B_TOTAL, S, D = 16, 577, 1408
H, HD = 16, 88
SCALE = HD ** -0.5
B = B_TOTAL // N_CORES          # batches per core = 2
T = B * S                       # tokens per core = 1154
SP = S + 1                      # padded per-batch token span = 578
KT = D // 128                   # 11 k-tiles over D
MT = 2 * KT                     # 22 m-tiles over the packed q|k blocks
TT = (S + 127) // 128           # 5 token tiles per batch (128,128,128,128,65)
VG = 97                         # v group width per head: 88 v cols + 9 ones
DEN = 96                        # psum partition of the softmax denominator

# moving-dim chunks for 578-wide spans: (logical col, width)
CH_S = [(0, 512), (512, 66)]
# chunks for 1408-wide spans
CH_D = [(0, 512), (512, 512), (1024, 384)]


def _tok_tiles():
    out = []
    for tt in range(TT):
        t0 = tt * 128
        out.append((tt, t0, min(128, S - t0)))
    return out


def build_program():
    nc = bacc.Bacc("TRN2", target_bir_lowering=False, debug=False,
                   num_devices=N_CORES)

    x_ap = nc.dram_tensor("x", [T, D], F32, kind="ExternalInput").ap()
    wqkv_ap = nc.dram_tensor("w_qkv_bf", [D, 3 * D], BF16, kind="ExternalInput").ap()
    bq_col_ap = nc.dram_tensor("b_qkv_col", [2 * D, 1], F32, kind="ExternalInput").ap()
    bv_row_ap = nc.dram_tensor("b_v_row", [1, D], BF16, kind="ExternalInput").ap()
    wp_ap = nc.dram_tensor("w_proj_bf", [D, D], BF16, kind="ExternalInput").ap()
    bp_row_ap = nc.dram_tensor("b_proj_row", [1, D], BF16, kind="ExternalInput").ap()
    ones_ap = nc.dram_tensor("ones_f", [1, 128], F32, kind="ExternalInput").ap()
    ones_bf_ap = nc.dram_tensor("ones_bf", [128, 128], BF16, kind="ExternalInput").ap()
    out_ap = nc.dram_tensor("out", [T, D], F32, kind="ExternalOutput").ap()

    with tile.TileContext(nc) as tc, contextlib.ExitStack() as ctx:
        p_xraw = ctx.enter_context(tc.tile_pool(name="xraw", bufs=2))
        p_xT = ctx.enter_context(tc.tile_pool(name="xT", bufs=11))
        p_vsb = ctx.enter_context(tc.tile_pool(name="vsb", bufs=5))
        p_qksb = ctx.enter_context(tc.tile_pool(name="qksb", bufs=6))
        p_qk = ctx.enter_context(tc.tile_pool(name="qk", bufs=34))
        p_expT = ctx.enter_context(tc.tile_pool(name="expT", bufs=8))
        p_pvsb = ctx.enter_context(tc.tile_pool(name="pvsb", bufs=3))
        p_attn = ctx.enter_context(tc.tile_pool(name="attn", bufs=6))
        p_apk = ctx.enter_context(tc.tile_pool(name="apk", bufs=12))
        p_rec = ctx.enter_context(tc.tile_pool(name="rec", bufs=3))
        p_wq = ctx.enter_context(tc.tile_pool(name="wq", bufs=4))
        p_wv = ctx.enter_context(tc.tile_pool(name="wv", bufs=13))
        p_wp = ctx.enter_context(tc.tile_pool(name="wp", bufs=17))
        p_bias = ctx.enter_context(tc.tile_pool(name="bias", bufs=8))
        p_brow = ctx.enter_context(tc.tile_pool(name="brow", bufs=2))
        p_const = ctx.enter_context(tc.tile_pool(name="const", bufs=1))
        p_ost = ctx.enter_context(tc.tile_pool(name="ost", bufs=3))

        psum = ctx.enter_context(tc.tile_pool(name="psum", bufs=8, space="PSUM"))

        def ps():
            return psum.tile([128, 512], F32, tag="ps", name="pst")

        ident = p_const.tile([128, 128], F32, tag="ident")
        make_identity(nc, ident[:])
        ones = p_const.tile([1, 128], F32R, tag="ones")
        nc.sync.dma_start(ones[:], ones_ap[0:1, :].bitcast(F32R))
        ones_bf = p_const.tile([1, 128], BF16, tag="ones_bf")
        nc.sync.dma_start(ones_bf[:], ones_bf_ap[0:1, :])

        bvr = p_brow.tile([1, D], BF16, tag="bvr")
        nc.sync.dma_start(bvr[:], bv_row_ap[:])
        bpr = p_brow.tile([1, D], BF16, tag="bpr")
        nc.sync.dma_start(bpr[:], bp_row_ap[:])

        # x^T tiles cover both batches; allocated once, written per batch.
        xT = [p_xT.tile([128, B * SP], BF16, tag="xT", name=f"xT{k}")
              for k in range(KT)]

        for b in range(B):
            boff = b * SP

            # ---- stage A: load x (token-major) and transpose to x^T ----
            for tt, t0, ts in _tok_tiles():
                xr = p_xraw.tile([128, D], F32, tag="xraw")
                nc.sync.dma_start(xr[0:ts, :], x_ap[b * S + t0: b * S + t0 + ts, :])
                for k in range(KT):
                    pt = ps()
                    nc.tensor.transpose(pt[0:128, 0:ts], xr[0:ts, k * 128:(k + 1) * 128],
                                        ident[0:ts, 0:ts])
                    nc.vector.tensor_copy(xT[k][:, boff + t0: boff + t0 + ts],
                                          pt[0:128, 0:ts])
            # fill padded token column (keeps downstream values finite)
            for k in range(KT):
                nc.sync.dma_start(xT[k][:, boff + S: boff + S + 1],
                                  ones_bf_ap[:, 0:1])

            # ---- stage B: v projection, token-major, head-interleaved ----
            vsb = []
            for tt, t0, ts in _tok_tiles():
                vt = p_vsb.tile([128, H * VG], BF16, tag="vsb")
                vsb.append(vt)
                nc.vector.memset(vt[:], 1.0)  # ones-columns (+ unused rows)
            for (c0, w) in CH_D:
                wvs = []
                for k in range(KT):
                    wv = p_wv.tile([128, 512], BF16, tag="wv")
                    nc.gpsimd.dma_start(
                        wv[:, 0:w],
                        wqkv_ap[k * 128:(k + 1) * 128, 2 * D + c0: 2 * D + c0 + w])
                    wvs.append(wv)
                for tt, t0, ts in _tok_tiles():
                    pv = ps()
                    for k in range(KT):
                        nc.tensor.matmul(pv[0:ts, 0:w],
                                         xT[k][:, boff + t0: boff + t0 + ts],
                                         wvs[k][:, 0:w], start=(k == 0), stop=False)
                    nc.tensor.matmul(pv[0:ts, 0:w], ones_bf[:, 0:ts],
                                     bvr[:, c0:c0 + w], start=False, stop=True)
                    # split per head into the 97-wide groups
                    h0 = c0 // HD
                    h1 = min(H - 1, (c0 + w - 1) // HD)
                    for h in range(h0, h1 + 1):
                        s0 = max(c0, h * HD)
                        s1 = min(c0 + w, (h + 1) * HD)
                        if s1 <= s0:
                            continue
                        nc.vector.tensor_copy(
                            vsb[tt][0:ts, h * VG + (s0 - h * HD): h * VG + (s1 - h * HD)],
                            pv[0:ts, s0 - c0: s1 - c0])

            # ---- stage C1: packed q|k projection (M=128 tiles) + head
            # redistribution via partition-shifting SBUF->SBUF DMAs ----
            qh = [None] * H
            kh = [None] * H
            frag = {}   # head tile -> next partition row to fill
            for m in range(MT):
                col = m * 128
                wq = p_wq.tile([128, KT * 128], BF16, tag="wq")
                nc.gpsimd.dma_start(
                    wq[:].rearrange("p (k c) -> p k c", k=KT),
                    wqkv_ap[:, col: col + 128].rearrange("(k p) c -> p k c", p=128))
                pts = []
                for (lc, w) in CH_S:
                    pt = ps()
                    for k in range(KT):
                        nc.tensor.matmul(pt[0:128, 0:w],
                                         wq[:, k * 128:(k + 1) * 128],
                                         xT[k][:, boff + lc: boff + lc + w],
                                         start=(k == 0), stop=(k == KT - 1))
                    pts.append(pt)
                bq = p_bias.tile([128, 1], F32, tag="bias")
                nc.sync.dma_start(bq[:], bq_col_ap[col: col + 128, :])
                qksb = p_qksb.tile([128, SP], BF16, tag="qksb")
                for (lc, w), pt in zip(CH_S, pts):
                    nc.scalar.activation(qksb[:, lc:lc + w], pt[0:128, 0:w],
                                         mybir.ActivationFunctionType.Identity,
                                         bias=bq[:])
                # ship finished head rows out of this m-tile
                which, dst = (0, qh) if m < KT else (1, kh)
                f_lo, f_hi = (m - which * KT) * 128, (m - which * KT) * 128 + 128
                for h in range(f_lo // HD, min(H, (f_hi + HD - 1) // HD)):
                    s0 = max(f_lo, h * HD)
                    s1 = min(f_hi, (h + 1) * HD)
                    if s1 <= s0:
                        continue
                    if dst[h] is None:
                        dst[h] = p_qk.tile([HD, SP], BF16, tag="qk",
                                           name=f"qk_{b}_{which}_{h}")
                    r0 = s0 - h * HD
                    nc.sync.dma_start(dst[h][r0: r0 + (s1 - s0), :],
                                      qksb[s0 - f_lo: s1 - f_lo, :])

            # ---- stage C2: per-head attention ----
            apk = [p_apk.tile([128, SP], BF16, tag="apk", name=f"apk_{b}_{k}")
                   for k in range(KT)]
            for h in range(H):
                # transposed scores + exp, per k-token tile
                expT = []
                for tt, t0, ts in _tok_tiles():
                    pts = []
                    for (lc, w) in CH_S:
                        pt = ps()
                        nc.tensor.matmul(pt[0:ts, 0:w],
                                         kh[h][:, t0:t0 + ts], qh[h][:, lc:lc + w],
                                         start=True, stop=True)
                        pts.append(pt)
                    et = p_expT.tile([128, SP], BF16, tag="expT")
                    expT.append(et)
                    for (lc, w), pt in zip(CH_S, pts):
                        nc.scalar.activation(et[0:ts, lc:lc + w], pt[0:ts, 0:w],
                                             mybir.ActivationFunctionType.Exp,
                                             scale=SCALE)

                # PV with fused denominator at psum partition 96
                pvs_ps = []
                for (lc, w) in CH_S:
                    pv = ps()
                    for tt, t0, ts in _tok_tiles():
                        nc.tensor.matmul(pv[0:VG, 0:w],
                                         vsb[tt][0:ts, h * VG:(h + 1) * VG],
                                         expT[tt][0:ts, lc:lc + w],
                                         start=(tt == 0), stop=(tt == TT - 1))
                    pvs_ps.append(pv)

                rec = p_rec.tile([1, SP], F32R, tag="rec")
                with nc.allow_low_precision(reason="softmax reciprocal"):
                    for (lc, w), pv in zip(CH_S, pvs_ps):
                        nc.vector.reciprocal(rec[:, lc:lc + w],
                                             pv[DEN:DEN + 1, 0:w])
                pvs = p_pvsb.tile([HD, SP], F32, tag="pvsb")
                for (lc, w), pv in zip(CH_S, pvs_ps):
                    nc.scalar.activation(pvs[:, lc:lc + w], pv[0:HD, 0:w],
                                         mybir.ActivationFunctionType.Copy)
                # broadcast 1/denom over partitions via rank-1 matmul
                at = p_attn.tile([HD, SP], BF16, tag="attn")
                for (lc, w), pv in zip(CH_S, pvs_ps):
                    pb = ps()
                    nc.tensor.matmul(pb[0:HD, 0:w], ones[:, 0:HD],
                                     rec[:, lc:lc + w], start=True, stop=True)
                    nc.vector.tensor_mul(at[:, lc:lc + w], pvs[:, lc:lc + w],
                                         pb[0:HD, 0:w])
                # pack into 128-wide K tiles for the output projection
                f0 = h * HD
                k0, r0 = f0 // 128, f0 % 128
                n0 = min(HD, 128 - r0)
                nc.sync.dma_start(apk[k0][r0: r0 + n0, :], at[0:n0, :])
                if n0 < HD:
                    nc.sync.dma_start(apk[k0 + 1][0: HD - n0, :], at[n0:HD, :])

            # ---- stage D: output projection (token-major, K=128 pieces) ----
            for (c0, w) in CH_D:
                wps = []
                for k in range(KT):
                    wpt = p_wp.tile([128, 512], BF16, tag="wp")
                    nc.gpsimd.dma_start(wpt[:, 0:w],
                                        wp_ap[k * 128:(k + 1) * 128, c0:c0 + w])
                    wps.append(wpt)
                for tt, t0, ts in _tok_tiles():
                    po = ps()
                    for k in range(KT):
                        nc.tensor.matmul(po[0:ts, 0:w], apk[k][:, t0:t0 + ts],
                                         wps[k][:, 0:w], start=(k == 0), stop=False)
                    nc.tensor.matmul(po[0:ts, 0:w], ones_bf[:, 0:ts],
                                     bpr[:, c0:c0 + w], start=False, stop=True)
                    ot = p_ost.tile([128, 512], F32, tag="ost")
                    nc.scalar.activation(ot[0:ts, 0:w], po[0:ts, 0:w],
                                         mybir.ActivationFunctionType.Copy)
                    nc.sync.dma_start(
                        out_ap[b * S + t0: b * S + t0 + ts, c0:c0 + w], ot[0:ts, 0:w])

    nc.compile()
    return nc


_NC_CACHE = None


def _get_nc():
    global _NC_CACHE
    if _NC_CACHE is None:
        _NC_CACHE = build_program()
    return _NC_CACHE


def make_in_maps(hidden_states, w_qkv, b_qkv, w_proj, b_proj):
    hidden_states = np.asarray(hidden_states, dtype=np.float32)
    w_qkv = np.ascontiguousarray(np.asarray(w_qkv, dtype=np.float32))
    b_qkv = np.asarray(b_qkv, dtype=np.float32)
    w_proj = np.asarray(w_proj, dtype=np.float32)
    b_proj = np.asarray(b_proj, dtype=np.float32)

    wqkv_bf = w_qkv.astype(ml_dtypes.bfloat16)
    wp_bf = w_proj.astype(ml_dtypes.bfloat16)
    bq_col = b_qkv[: 2 * D].reshape(2 * D, 1).copy()
    bv_row = b_qkv[2 * D:].astype(ml_dtypes.bfloat16).reshape(1, D).copy()
    bp_row = b_proj.astype(ml_dtypes.bfloat16).reshape(1, D).copy()
    ones_f = np.ones((1, 128), np.float32)
    ones_bf = np.ones((128, 128), ml_dtypes.bfloat16)

    in_maps = []
    for c in range(N_CORES):
        xs = hidden_states[c * B:(c + 1) * B].reshape(T, D)
        in_maps.append({
            "x": np.ascontiguousarray(xs),
            "w_qkv_bf": wqkv_bf,
            "b_qkv_col": bq_col,
            "b_v_row": bv_row,
            "w_proj_bf": wp_bf,
            "b_proj_row": bp_row,
            "ones_f": ones_f,
            "ones_bf": ones_bf,
        })
    return in_maps


def kernel(hidden_states, w_qkv, b_qkv, w_proj, b_proj):
    nc = _get_nc()
    in_maps = make_in_maps(hidden_states, w_qkv, b_qkv, w_proj, b_proj)
    res = run_bass_kernel_spmd(nc, in_maps, list(range(N_CORES)))
    out = np.concatenate(
        [res.results[c]["out"].reshape(B, S, D) for c in range(N_CORES)], axis=0)
    return out.astype(np.float32)


if __name__ == "__main__":
    rng = np.random.default_rng(0)
    hs = rng.standard_normal((B_TOTAL, S, D), dtype=np.float32)
    wq = rng.standard_normal((D, 3 * D), dtype=np.float32) * D ** -0.5
    bq = rng.standard_normal(3 * D).astype(np.float32) * 0.02
    wp = rng.standard_normal((D, D), dtype=np.float32) * D ** -0.5
    bp = rng.standard_normal(D).astype(np.float32) * 0.02
    o = kernel(hidden_states=hs, w_qkv=wq, b_qkv=bq, w_proj=wp, b_proj=bp)
    print(o.shape, o.dtype)
